# revision 1
# baseline (speedup 1.0000x reference)
"""Trainium2 Bass kernel for single-head attention (AutoCorrelationLayer).

Full-input contract: kernel(**inputs) takes the unsharded inputs
  x [8, 2048, 1024], Wq/Wk/Wv [1024, 1024], bq/bk/bv [1024]
and returns y [8, 2048, 1024].

Sharding: data-parallel over batch — one batch element per NeuronCore
(B == n_cores == 8). Weights/biases are replicated to every core. Each core
runs the same single-core Bass program; no collectives.

Per-core dataflow (S=2048, D=1024; all matmul-path tensors are native
float32r, which runs the 128x128 PE at full rate — the BIR verifier requires
producers to round to fp32r, so the dtype is carried end-to-end). Phases are
arranged so per-phase DMA stays below per-phase PE work (HBM is one shared
~360 GB/s pipe):
  Phase AV (fused): per 256-column s-chunk, transpose x to d-major on the PE
           (chunk kept in SBUF, also stored to a blocked xT DRAM scratch for
           the QK phase), and compute V = x @ Wv + bv into SBUF using the
           transposed chunk as the stationary operand. The V matmuls trail
           the transposes by one chunk so PSUM->SBUF copies never stall PE.
  Phase QK: Q and K projections from the xT scratch with both weight
           matrices resident; QT + bq -> blocked qT DRAM scratch,
           KT + bk -> SBUF resident.
  Phase D: per 128-row q-block, software-pipelined on the PE
           (scores(i+1) | P-transposes(i), PV(i)):
           scores = QT_blk^T @ KT into PSUM [128, S]; exp + row-sum in one
           ACT instruction (no max-subtraction: |logits| <= ~7 here, and
           softmax is shift-invariant); P^T via PE transposes;
           out = P^T^T @ V; scale rows by 1/l; DMA out.
  DMA rings: loads on SP (nc.sync); weights and stores on ACT (nc.scalar);
  the bias broadcast on SWDGE (nc.gpsimd). Weights are split so the first
  consumer groups never wait for a full 4 MB DMA.
"""

from contextlib import ExitStack

import numpy as np

import concourse.bacc as bacc
import concourse.bass as bass
import concourse.mybir as mybir
import concourse.tile as tile
from concourse.bass_utils import run_bass_kernel_spmd
from concourse.masks import make_identity

F32 = mybir.dt.float32
F32R = mybir.dt.float32r
AFT = mybir.ActivationFunctionType
AX = mybir.AxisListType
P = 128

B, S, D = 8, 2048, 1024
N_CORES = 8


def build_attention_nc(S=2048, D=1024, reps=1):
    nc = bacc.Bacc(dynamic_dma_scratch_size=4096)
    DC = D // P      # d chunks (8)
    SB = S // P      # s blocks (16)
    SC = 256         # s-chunk width for AV/QK phases
    NSC = S // SC
    KCH = 512        # moving-dim chunk for score/PV matmuls
    scale = 1.0 / float(D) ** 0.5

    x = nc.dram_tensor("x", [S, D], F32R, kind="ExternalInput")
    Wq = nc.dram_tensor("Wq", [D, D], F32R, kind="ExternalInput")
    Wk = nc.dram_tensor("Wk", [D, D], F32R, kind="ExternalInput")
    Wv = nc.dram_tensor("Wv", [D, D], F32R, kind="ExternalInput")
    bq = nc.dram_tensor("bq", [D], F32, kind="ExternalInput")
    bk = nc.dram_tensor("bk", [D], F32, kind="ExternalInput")
    bv = nc.dram_tensor("bv", [D], F32, kind="ExternalInput")
    y = nc.dram_tensor("y", [S, D], F32, kind="ExternalOutput")
    # chunk-blocked scratch layouts: [chunk][partition][d-chunk][s-in-chunk]
    # so chunk stores/loads are fully contiguous per partition row (8 KB)
    xT_dram = nc.dram_tensor("xT_scratch", [S // SC, P, DC, SC], F32R)
    qT_dram = nc.dram_tensor("qT_scratch", [S // SC, P, DC, SC], F32R)

    def load_w(w_sb, W, e0, e1, eng=None):
        # per-ci-chunk DMAs so consumers can start before the whole matrix lands
        wr = W.rearrange("(c p) e -> p c e", p=P)
        for ci in range(DC):
            (eng or nc.scalar).dma_start(out=w_sb[:, ci, :], in_=wr[:, ci, e0:e1])

    def proj_matmuls(ps, w_ap, xt_ap, c):
        for ci in range(DC):
            nc.tensor.matmul(ps, w_ap[:, ci, c * P:(c + 1) * P], xt_ap[:, ci, :],
                             start=(ci == 0), stop=(ci == DC - 1))

    with tile.TileContext(nc) as tc, ExitStack() as ctx:
        persist = ctx.enter_context(tc.tile_pool(name="persist", bufs=1))
        KT_sb = persist.tile([P, DC, S], F32R, tag="KT")
        V_sb = persist.tile([P, SB, D], F32R, tag="V")
        ident32 = persist.tile([P, P], F32, tag="ident32")
        make_identity(nc, ident32)
        ident = persist.tile([P, P], F32R, tag="ident")
        nc.vector.tensor_copy(ident, ident32)
        bq_sb = persist.tile([P, DC], F32, tag="bq")
        bk_sb = persist.tile([P, DC], F32, tag="bk")
        bv_sb = persist.tile([P, D], F32, tag="bv")
        nc.gpsimd.dma_start(out=bq_sb, in_=bq.rearrange("(c p) -> p c", p=P))
        nc.gpsimd.dma_start(out=bk_sb, in_=bk.rearrange("(c p) -> p c", p=P))
        nc.gpsimd.dma_start(out=bv_sb, in_=bv[:].partition_broadcast(P))

        for _rep in range(reps):
            # Wq low half prefetched during phase AV (Q groups c<4 need only it)
            with tc.tile_pool(name="wqlo", bufs=1) as wqlop:
                Wq_lo = wqlop.tile([P, DC, KCH], F32R, tag="Wqlo")

                # ---- Phase AV: x -> xT (PE transpose) + V projection ----
                with nc.named_scope("phaseAV"), \
                     tc.tile_pool(name="wvlo", bufs=1) as wvlop, \
                     tc.tile_pool(name="wvhi", bufs=1) as wvhip, \
                     tc.tile_pool(name="ax", bufs=2) as axp, \
                     tc.tile_pool(name="astg", bufs=2) as astgp, \
                     tc.tile_pool(name="apsum", bufs=4, space="PSUM") as apsp, \
                     tc.tile_pool(name="vps", bufs=4, space="PSUM") as vpsp:
                    Wv_lo = wvlop.tile([P, DC, KCH], F32R, tag="Wvlo")
                    Wv_hi = wvhip.tile([P, DC, KCH], F32R, tag="Wvhi")
                    load_w(Wv_lo, Wv, 0, KCH)
                    load_w(Wv_hi, Wv, KCH, D)
                    wv_half = (Wv_lo, Wv_hi)

                    def emit_v_chunk(sc, stg):
                        for h in range(D // KCH):
                            for j in range(SC // P):
                                kb = sc * (SC // P) + j
                                ps = vpsp.tile([P, KCH], F32, tag="ps")
                                for ci in range(DC):
                                    nc.tensor.matmul(
                                        ps, stg[:, ci, j * P:(j + 1) * P],
                                        wv_half[h][:, ci, :],
                                        start=(ci == 0), stop=(ci == DC - 1))
                                nc.vector.tensor_add(
                                    V_sb[:, kb, h * KCH:(h + 1) * KCH], ps,
                                    bv_sb[:, h * KCH:(h + 1) * KCH])

                    prev = None
                    for sc in range(NSC):
                        x_ts = []
                        for j in range(SC // P):
                            x_t = axp.tile([P, D], F32R, tag=f"x_t{j}")
                            nc.sync.dma_start(
                                out=x_t,
                                in_=x[sc * SC + j * P: sc * SC + (j + 1) * P, :])
                            x_ts.append(x_t)
                        stg = astgp.tile([P, DC, SC], F32R, tag="stg")
                        for c in range(DC):
                            pst = apsp.tile([P, SC // P, P], F32R, tag="pst")
                            for j in range(SC // P):
                                nc.tensor.transpose(
                                    pst[:, j, :], x_ts[j][:, c * P:(c + 1) * P],
                                    ident)
                            nc.vector.tensor_copy(stg[:, c, :], pst)
                        nc.scalar.dma_start(out=xT_dram[sc], in_=stg)
                        if prev is not None:
                            emit_v_chunk(sc - 1, prev)
                        prev = stg
                        if sc == 4:
                            # prefetch for phase QK
                            load_w(Wq_lo, Wq, 0, KCH)
                    emit_v_chunk(NSC - 1, prev)

                # ---- Phase QK: Q/K projections from xT scratch ----
                with nc.named_scope("phaseQK"), \
                     tc.tile_pool(name="wqhi", bufs=1) as wqhip, \
                     tc.tile_pool(name="wk", bufs=1) as wkp, \
                     tc.tile_pool(name="qkxt", bufs=2) as xtp, \
                     tc.tile_pool(name="qstg", bufs=2) as qstgp, \
                     tc.tile_pool(name="qkps", bufs=8, space="PSUM") as qkpsp:
                    Wq_hi = wqhip.tile([P, DC, KCH], F32R, tag="Wqhi")
                    Wk_sb = wkp.tile([P, DC, D], F32R, tag="Wk")
                    load_w(Wq_hi, Wq, KCH, D)
                    load_w(Wk_sb, Wk, 0, D)
                    wq_half = (Wq_lo, Wq_hi)

                    for sc in range(NSC):
                        xt_t = xtp.tile([P, DC, SC], F32R, tag="xt_t")
                        nc.sync.dma_start(out=xt_t, in_=xT_dram[sc])
                        for c in range(DC):
                            ps = qkpsp.tile([P, SC], F32, tag="ps")
                            proj_matmuls(ps, wq_half[c // 4], xt_t, c % 4)
                            qstg = qstgp.tile([P, SC], F32R, tag="stg")
                            nc.scalar.activation(qstg, ps, AFT.Identity,
                                                 bias=bq_sb[:, c:c + 1],
                                                 scale=1.0)
                            nc.scalar.dma_start(out=qT_dram[sc, :, c, :],
                                                in_=qstg)
                        for c in range(DC):
                            ps = qkpsp.tile([P, SC], F32, tag="ps")
                            proj_matmuls(ps, Wk_sb, xt_t, c)
                            nc.vector.tensor_scalar_add(
                                KT_sb[:, c, sc * SC:(sc + 1) * SC], ps,
                                bk_sb[:, c:c + 1])

            # ---- Phase D: attention, software-pipelined over q-blocks ----
            with nc.named_scope("phaseD"), \
                 tc.tile_pool(name="dqt", bufs=2) as dqtp, \
                 tc.tile_pool(name="dp", bufs=2) as dpp, \
                 tc.tile_pool(name="dpt", bufs=2) as dptp, \
                 tc.tile_pool(name="do", bufs=2) as dop, \
                 tc.tile_pool(name="dst", bufs=8) as dstp, \
                 tc.tile_pool(name="dpsS", bufs=1, space="PSUM") as psS, \
                 tc.tile_pool(name="dpsT", bufs=2, space="PSUM") as psT, \
                 tc.tile_pool(name="dpsO", bufs=1, space="PSUM") as psO:

                qt_pre = {}

                def fetch_qt(qc):
                    qt_t = dqtp.tile([P, DC, SC], F32R, tag="qt_t")
                    nc.sync.dma_start(out=qt_t, in_=qT_dram[qc])
                    return qt_t

                def emit_scores(qb):
                    qc, qo = divmod(qb, SC // P)
                    qt_c = qt_pre.pop(qc, None)
                    if qt_c is None:
                        qt_c = fetch_qt(qc)
                    if qo + 1 < SC // P:
                        qt_pre[qc] = qt_c
                    qt_t = qt_c[:, :, qo * P:(qo + 1) * P]
                    ps_s = psS.tile([P, S], F32, tag="ps_s")
                    for k4 in range(S // KCH):
                        for c in range(DC):
                            nc.tensor.matmul(
                                ps_s[:, k4 * KCH:(k4 + 1) * KCH],
                                qt_t[:, c, :],
                                KT_sb[:, c, k4 * KCH:(k4 + 1) * KCH],
                                start=(c == 0), stop=(c == DC - 1))
                    # no max-subtraction: |logits| <= ~7 for this problem,
                    # exp is safe in fp32 and softmax is shift-invariant.
                    p_t = dpp.tile([P, S], F32R, tag="p_t")
                    l_t = dstp.tile([P, 1], F32, tag="l_t")
                    nc.scalar.activation(p_t, ps_s, AFT.Exp, bias=0.0,
                                         scale=scale, accum_out=l_t)
                    return p_t, l_t

                def emit_pv(p_t, l_t, qb):
                    pt_t = dptp.tile([P, SB, P], F32R, tag="pt_t")
                    for g in range(SB // 4):
                        pst = psT.tile([P, 4, P], F32R, tag="pst")
                        for t in range(4):
                            kb = g * 4 + t
                            nc.tensor.transpose(
                                pst[:, t, :], p_t[:, kb * P:(kb + 1) * P], ident)
                        nc.vector.tensor_copy(pt_t[:, g * 4:(g + 1) * 4, :], pst)
                    rl = dstp.tile([P, 1], F32, tag="rl")
                    nc.vector.reciprocal(rl, l_t)
                    ps_o = psO.tile([P, D], F32, tag="ps_o")
                    for h in range(D // KCH):
                        for kb in range(SB):
                            nc.tensor.matmul(
                                ps_o[:, h * KCH:(h + 1) * KCH],
                                pt_t[:, kb, :],
                                V_sb[:, kb, h * KCH:(h + 1) * KCH],
                                start=(kb == 0), stop=(kb == SB - 1))
                    o_t = dop.tile([P, D], F32, tag="o_t")
                    nc.vector.tensor_scalar_mul(o_t, ps_o, rl)
                    nc.scalar.dma_start(out=y[qb * P:(qb + 1) * P, :], in_=o_t)

                prev = None
                for qb in range(SB):
                    cur = emit_scores(qb)
                    qc_next = qb // (SC // P) + 1
                    if qb % (SC // P) == 0 and qc_next < NSC \
                            and qc_next not in qt_pre:
                        qt_pre[qc_next] = fetch_qt(qc_next)
                    if prev is not None:
                        emit_pv(*prev)
                    prev = (*cur, qb)
                emit_pv(*prev)

    nc.compile()
    return nc


_NC_CACHE = {}


def _get_nc():
    if "nc" not in _NC_CACHE:
        _NC_CACHE["nc"] = build_attention_nc(S=S, D=D)
    return _NC_CACHE["nc"]


def run(inputs, trace=False, **run_kwargs):
    """Shard over batch, run on cores 0..7, gather. Returns (y, BassKernelResults)."""
    x = np.ascontiguousarray(np.asarray(inputs["x"], dtype=np.float32))
    shared = {
        k: np.ascontiguousarray(np.asarray(inputs[k], dtype=np.float32))
        for k in ("Wq", "Wk", "Wv", "bq", "bk", "bv")
    }
    in_maps = [dict(shared, x=x[b]) for b in range(B)]
    nc = _get_nc()
    res = run_bass_kernel_spmd(nc, in_maps, core_ids=list(range(N_CORES)),
                               trace=trace, **run_kwargs)
    y = np.stack([res.results[b]["y"] for b in range(B)], axis=0)
    return y, res


def kernel(**inputs):
    y, _ = run(inputs, trace=False)
    return y



# revision 2
# speedup vs baseline: 781.9329x; 781.9329x over previous
"""Trainium2 Bass kernel for single-head attention (AutoCorrelationLayer), v3.

Full-input contract: kernel(**inputs) takes the unsharded inputs
  x [8, 2048, 1024], Wq/Wk/Wv [1024, 1024], bq/bk/bv [1024]
and returns y [8, 2048, 1024].

Sharding: data-parallel over batch — one batch element per NeuronCore
(B == n_cores == 8). Weights/biases are replicated; no collectives.

v3 = all-bf16 matmul path, transposed scores, no DRAM scratch, and a fully
PERSISTENT SBUF/PSUM layout: every tile and pool is allocated once, outside
the rep loop, so rep k+1's DMAs and bf16 conversions prefetch during rep
k's stage 2 and the in-order PE stream crosses rep seams without stalling.

Memory plan (per partition): persistent xT/qT/KT/V (4x32K) + Wv bf16 (16K)
+ P^T (16K) + staging ~25K ≈ 191K of ~208K SBUF. Wq/Wk stream through
small per-column-slice pools (2K each, double-buffered, self-pacing via
pool-generation WAR). PSUM: transpose bank (1) + shared work pool for
projections/scores/row-sums (3x512-col banks) + PV accumulators (2x2
banks) = 8 banks exactly.

Engines: PE matmuls only; DVE x-conversions, drains, output scale; ACT
weight conversions + exp; SP (sync) ring carries all loads in exact
PE-consumption order [x0,x1, Wv-lo, x2,x3, Wv-hi, x4..15, Wq-slices,
Wk-slices]; y stores ride the otherwise-idle ACT ring so they never block
the sync ring's cross-rep prefetch.

Per-core dataflow (S=2048, D=1024):
  Stage 1: per 128-row block: x fp32 -> DVE cvt bf16 -> PE transpose ->
    DVE drain into resident xT. V h=0 groups (Wv-lo moving) interleave
    behind the transposes, V h=1 into the Q/K phases. qT/KT projections
    run c-outer consuming one 128-col weight slice at a time; bias added
    on the DVE drain. Softmax denominators: see stage 2.
  Stage 2 (per 512-col q-group): sT = KT-chunk @ qT into a work-pool
    PSUM generation, ACT exp(scale=1/32) -> P^T bf16 (single buffer);
    PV per q-block: P^T chunks stationary, V moving, plus a ones-column
    matmul accumulating softmax row sums into a work-pool generation;
    DVE reciprocal + per-half scale; y per 512-col half. Softmax skips
    max-subtraction (|logits| <= ~7, exp safe in fp32, shift-invariant).
"""

from contextlib import ExitStack

import numpy as np

import concourse.bacc as bacc
import concourse.bass as bass
import concourse.mybir as mybir
import concourse.tile as tile
from concourse.bass_utils import run_bass_kernel_spmd
from concourse.masks import make_identity

F32 = mybir.dt.float32
BF16 = mybir.dt.bfloat16
AFT = mybir.ActivationFunctionType
P = 128

B, S, D = 8, 2048, 1024
N_CORES = 8


def build_attention_nc(S=2048, D=1024, reps=1):
    nc = bacc.Bacc(dynamic_dma_scratch_size=4096)
    DC = D // P      # d/e chunks (8)
    SB = S // P      # s blocks (16)
    PG = 512         # projection span / stage-2 q-group width
    NQG = S // PG    # stage-2 q groups (4)
    scale = 1.0 / float(D) ** 0.5

    x = nc.dram_tensor("x", [S, D], F32, kind="ExternalInput")
    Wq = nc.dram_tensor("Wq", [D, D], F32, kind="ExternalInput")
    Wk = nc.dram_tensor("Wk", [D, D], F32, kind="ExternalInput")
    Wv = nc.dram_tensor("Wv", [D, D], F32, kind="ExternalInput")
    bq = nc.dram_tensor("bq", [D], F32, kind="ExternalInput")
    bk = nc.dram_tensor("bk", [D], F32, kind="ExternalInput")
    bv = nc.dram_tensor("bv", [D], F32, kind="ExternalInput")
    y = nc.dram_tensor("y", [S, D], F32, kind="ExternalOutput")

    with tile.TileContext(nc) as tc, ExitStack() as ctx:
        persist = ctx.enter_context(tc.tile_pool(name="persist", bufs=1))
        ident32 = persist.tile([P, P], F32, tag="ident32")
        make_identity(nc, ident32)
        ident = persist.tile([P, P], BF16, tag="ident")
        nc.vector.tensor_copy(ident, ident32)
        ones = persist.tile([P, 1], BF16, tag="ones")
        nc.gpsimd.memset(ones, 1.0)
        bq_sb = persist.tile([P, DC], F32, tag="bq")
        bk_sb = persist.tile([P, DC], F32, tag="bk")
        bv_sb = persist.tile([P, D], BF16, tag="bv")
        bv_f32 = persist.tile([P, D], F32, tag="bvf")
        nc.gpsimd.dma_start(out=bq_sb, in_=bq.rearrange("(c p) -> p c", p=P))
        nc.gpsimd.dma_start(out=bk_sb, in_=bk.rearrange("(c p) -> p c", p=P))
        nc.gpsimd.dma_start(out=bv_f32, in_=bv[:].partition_broadcast(P))
        nc.vector.tensor_copy(bv_sb, bv_f32)

        # ---- persistent big tensors ----
        qT_sb = persist.tile([P, DC, S], BF16, tag="qT")
        KT_sb = persist.tile([P, DC, S], BF16, tag="KT")
        V_sb = persist.tile([P, SB, D], BF16, tag="V")
        xT_sb = persist.tile([P, DC, S], BF16, tag="xT")
        Wv_sb = persist.tile([P, DC, D], BF16, tag="Wv")

        # ---- persistent pools ----
        xfp = ctx.enter_context(tc.tile_pool(name="xf", bufs=3))
        x16p = ctx.enter_context(tc.tile_pool(name="x16", bufs=2))
        wstgp = ctx.enter_context(tc.tile_pool(name="wstg", bufs=2))
        wslp = ctx.enter_context(tc.tile_pool(name="wsl", bufs=2))
        wqkp = ctx.enter_context(tc.tile_pool(name="wqk", bufs=2))
        ptp = ctx.enter_context(tc.tile_pool(name="pt", bufs=1))
        otp = ctx.enter_context(tc.tile_pool(name="ot", bufs=2))
        lstp = ctx.enter_context(tc.tile_pool(name="lst", bufs=4))
        tpsp = ctx.enter_context(tc.tile_pool(name="tps", bufs=1, space="PSUM"))
        workp = ctx.enter_context(tc.tile_pool(name="work", bufs=3, space="PSUM"))
        opsp = ctx.enter_context(tc.tile_pool(name="ops", bufs=2, space="PSUM"))

        def emit_rep():
            # ---- load emission on the sync ring, in PE-need order ----
            xf_tiles = {}

            def emit_x_load(sb):
                xf = xfp.tile([P, D], F32, tag="xf")
                nc.sync.dma_start(out=xf, in_=x[sb * P:(sb + 1) * P, :])
                xf_tiles[sb] = xf

            wvr = Wv.rearrange("(c p) e -> p c e", p=P)

            def load_wv_half(h):
                for ci in range(DC):
                    wst = wstgp.tile([P, PG], F32, tag="wst")
                    nc.sync.dma_start(
                        out=wst, in_=wvr[:, ci, h * PG:(h + 1) * PG])
                    nc.scalar.activation(
                        Wv_sb[:, ci, h * PG:(h + 1) * PG], wst,
                        AFT.Identity, scale=1.0)

            def load_w_slice(W, c):
                # one 128-col e-slice of Wq/Wk (all d chunks), bf16
                wr = W.rearrange("(c p) e -> p c e", p=P)
                wst = wslp.tile([P, DC, P], F32, tag="wsl")
                nc.sync.dma_start(out=wst, in_=wr[:, :, c * P:(c + 1) * P])
                wt = wqkp.tile([P, DC, P], BF16, tag="wqk")
                nc.scalar.activation(wt, wst, AFT.Identity, scale=1.0)
                return wt

            emit_x_load(0)
            emit_x_load(1)
            load_wv_half(0)
            emit_x_load(2)
            emit_x_load(3)
            load_wv_half(1)
            for sb in range(4, SB):
                emit_x_load(sb)

            # ---- Stage 1 ----
            with nc.named_scope("stage1"):
                # single PSUM generation per rep; slice-level hazards
                # pipeline the drains
                pst = tpsp.tile([P, DC, P], BF16, tag="pst")

                def emit_transpose(sb):
                    x16 = x16p.tile([P, D], BF16, tag="x16")
                    nc.vector.tensor_copy(x16, xf_tiles.pop(sb))
                    for g in range(2):
                        for c4 in range(4):
                            c = g * 4 + c4
                            nc.tensor.transpose(
                                pst[:, c, :], x16[:, c * P:(c + 1) * P],
                                ident)
                        nc.vector.tensor_copy(
                            xT_sb[:, g * 4:g * 4 + 4, sb * P:(sb + 1) * P],
                            pst[:, g * 4:g * 4 + 4, :])

                def emit_v(sb, h):
                    ps = workp.tile([P, PG], F32, tag="wps")
                    for ci in range(DC):
                        nc.tensor.matmul(
                            ps, xT_sb[:, ci, sb * P:(sb + 1) * P],
                            Wv_sb[:, ci, h * PG:(h + 1) * PG],
                            start=(ci == 0), stop=(ci == DC - 1))
                    nc.vector.tensor_add(
                        V_sb[:, sb, h * PG:(h + 1) * PG], ps,
                        bv_sb[:, h * PG:(h + 1) * PG])

                # V h=0 groups fill PE gaps behind the transposes (Wv-lo
                # lands by x2, so the V pop is never the blocking
                # instruction past block 2); h=1 groups spill into the
                # Q/K phases.
                for sb in range(SB):
                    emit_transpose(sb)
                    if sb >= 2:
                        emit_v(sb - 2, 0)
                vq = [(SB - 2, 0), (SB - 1, 0)]
                vq += [(sb, 1) for sb in range(SB)]

                def emit_proj(wt, out_sb, b_sb, c, g):
                    ps = workp.tile([P, PG], F32, tag="wps")
                    for ci in range(DC):
                        nc.tensor.matmul(
                            ps, wt[:, ci, :],
                            xT_sb[:, ci, g * PG:(g + 1) * PG],
                            start=(ci == 0), stop=(ci == DC - 1))
                    # drain on ACT (bias is per-partition in the e-major
                    # layout) to keep DVE off the stage-1 critical path
                    nc.scalar.activation(
                        out_sb[:, c, g * PG:(g + 1) * PG], ps,
                        AFT.Identity, bias=b_sb[:, c:c + 1], scale=1.0)

                # c-outer: one weight slice live at a time (wqk pool
                # generations self-pace the slice DMAs)
                for W, out_sb, b_sb in ((Wq, qT_sb, bq_sb),
                                        (Wk, KT_sb, bk_sb)):
                    for c in range(DC):
                        wt = load_w_slice(W, c)
                        for g in range(S // PG):
                            emit_proj(wt, out_sb, b_sb, c, g)
                            if vq:
                                emit_v(*vq.pop(0))
                while vq:
                    emit_v(*vq.pop(0))

            # ---- Stage 2 ----
            with nc.named_scope("stage2"):
                for qg in range(NQG):
                    PT = ptp.tile([P, SB, PG], BF16, tag="PT")
                    q_lo = qg * PG
                    for kb in range(SB):
                        ps = workp.tile([P, PG], F32, tag="wps")
                        for c in range(DC):
                            nc.tensor.matmul(
                                ps, KT_sb[:, c, kb * P:(kb + 1) * P],
                                qT_sb[:, c, q_lo:q_lo + PG],
                                start=(c == 0), stop=(c == DC - 1))
                        nc.scalar.activation(PT[:, kb, :], ps, AFT.Exp,
                                             bias=0.0, scale=scale)
                    for qb in range(PG // P):
                        q0 = q_lo + qb * P
                        ps_o = opsp.tile([P, D], F32, tag="ps_o")
                        lt = workp.tile([P, PG], F32, tag="wps")
                        ps_l = lt[:, 0:1]
                        for kb in range(SB):
                            pt_s = PT[:, kb, qb * P:(qb + 1) * P]
                            for h in range(D // PG):
                                nc.tensor.matmul(
                                    ps_o[:, h * PG:(h + 1) * PG], pt_s,
                                    V_sb[:, kb, h * PG:(h + 1) * PG],
                                    start=(kb == 0), stop=(kb == SB - 1))
                            nc.tensor.matmul(ps_l, pt_s, ones,
                                             start=(kb == 0),
                                             stop=(kb == SB - 1))
                        rl = lstp.tile([P, 1], F32, tag="rl")
                        nc.vector.reciprocal(rl, ps_l)
                        for h in range(D // PG):
                            sl = slice(h * PG, (h + 1) * PG)
                            o_t = otp.tile([P, PG], F32, tag="o_t")
                            nc.vector.tensor_scalar_mul(o_t, ps_o[:, sl], rl)
                            nc.scalar.dma_start(out=y[q0:q0 + P, sl], in_=o_t)

        for _rep in range(reps):
            emit_rep()

    nc.compile()
    return nc


_NC_CACHE = {}


def _get_nc():
    if "nc" not in _NC_CACHE:
        _NC_CACHE["nc"] = build_attention_nc(S=S, D=D)
    return _NC_CACHE["nc"]


def run(inputs, trace=False, **run_kwargs):
    """Shard over batch, run on cores 0..7, gather. Returns (y, BassKernelResults)."""
    x = np.ascontiguousarray(np.asarray(inputs["x"], dtype=np.float32))
    shared = {
        k: np.ascontiguousarray(np.asarray(inputs[k], dtype=np.float32))
        for k in ("Wq", "Wk", "Wv", "bq", "bk", "bv")
    }
    in_maps = [dict(shared, x=x[b]) for b in range(B)]
    nc = _get_nc()
    res = run_bass_kernel_spmd(nc, in_maps, core_ids=list(range(N_CORES)),
                               trace=trace, **run_kwargs)
    y = np.stack([res.results[b]["y"] for b in range(B)], axis=0)
    return y, res


def kernel(**inputs):
    y, _ = run(inputs, trace=False)
    return y


# revision 3
# speedup vs baseline: 811.3457x; 1.0376x over previous
"""Trainium2 Bass kernel for single-head attention (AutoCorrelationLayer), v4.

v4 = v3 (all-bf16, transposed scores, persistent SBUF/PSUM, cross-rep
prefetch) + score reassociation:
    scores = (xWq + bq)(xWk + bk)^T
           = x (Wq Wk^T) x^T  +  [const over keys]  +  (x Wk bq)^T 1
so with A := Wq Wk^T (D^3/2 MACs, 65.5K PE cycles) the K-projection
(131K cycles) disappears; the surviving bias term w := x Wk bq /sqrt(D)
rides the per-partition bias input of the exp activation (keys are the
partition dim in the transposed-scores layout), and the other terms
cancel under softmax shift-invariance. Net PE: ~887K cycles vs ~934K.

Pipeline per rep:
  Phase 0: Wq/Wk stream in 128-col e-slices (fp32->bf16 on ACT); PE
    transposes them into WqT (full) / WkT (d'-half at a time); A = WqT^T
    chunks @ WkT halves accumulates in PSUM, drains bf16. t := Wk^T-
    chunks^T @ bq-chunks (rank-1, N=1 matmuls) -> t16.
  Stage 1: x blocks: fp32 -> bf16 (DVE) -> PE transpose -> xT resident.
    B^T = A-chunks^T @ xT (e-major, replaces qT), per 512-col g-span,
    interleaved behind the transposes; V = xT-slices^T @ Wv likewise;
    w = xT-slices^T @ t16 (N=1 chain) with the 1/sqrt(D) scale folded
    into the ACT drain.
  Stage 2: per 512-col q-group: sT = xT-chunk^T @ B^T + exp with
    bias=w[key-slice] -> P^T; PV + ones-column row sums; reciprocal,
    scale, store. No max-subtraction (|logits| <= ~7).

Sharding: data-parallel over batch, one element per core, as before.
"""

from contextlib import ExitStack

import numpy as np

import concourse.bacc as bacc
import concourse.bass as bass
import concourse.mybir as mybir
import concourse.tile as tile
from concourse.bass_utils import run_bass_kernel_spmd
from concourse.masks import make_identity

F32 = mybir.dt.float32
BF16 = mybir.dt.bfloat16
AFT = mybir.ActivationFunctionType
P = 128

B, S, D = 8, 2048, 1024
N_CORES = 8


def build_attention_nc(S=2048, D=1024, reps=1):
    nc = bacc.Bacc(dynamic_dma_scratch_size=4096)
    DC = D // P      # d/e chunks (8)
    SB = S // P      # s blocks (16)
    PG = 512         # projection span / stage-2 q-group width
    NQG = S // PG    # stage-2 q groups (4)
    HDC = DC // 2    # d'-half chunk count (4)
    scale = 1.0 / float(D) ** 0.5

    x = nc.dram_tensor("x", [S, D], F32, kind="ExternalInput")
    Wq = nc.dram_tensor("Wq", [D, D], F32, kind="ExternalInput")
    Wk = nc.dram_tensor("Wk", [D, D], F32, kind="ExternalInput")
    Wv = nc.dram_tensor("Wv", [D, D], F32, kind="ExternalInput")
    bq = nc.dram_tensor("bq", [D], F32, kind="ExternalInput")
    bk = nc.dram_tensor("bk", [D], F32, kind="ExternalInput")
    bv = nc.dram_tensor("bv", [D], F32, kind="ExternalInput")
    y = nc.dram_tensor("y", [S, D], F32, kind="ExternalOutput")

    with tile.TileContext(nc) as tc, ExitStack() as ctx:
        persist = ctx.enter_context(tc.tile_pool(name="persist", bufs=1))
        ident32 = persist.tile([P, P], F32, tag="ident32")
        make_identity(nc, ident32)
        ident = persist.tile([P, P], BF16, tag="ident")
        nc.vector.tensor_copy(ident, ident32)
        ones = persist.tile([P, 1], BF16, tag="ones")
        nc.gpsimd.memset(ones, 1.0)
        bq_sb = persist.tile([P, DC], F32, tag="bq")
        bq16 = persist.tile([P, DC], BF16, tag="bq16")
        bv_sb = persist.tile([P, D], BF16, tag="bv")
        bv_f32 = persist.tile([P, D], F32, tag="bvf")
        nc.gpsimd.dma_start(out=bq_sb, in_=bq.rearrange("(c p) -> p c", p=P))
        nc.vector.tensor_copy(bq16, bq_sb)
        nc.gpsimd.dma_start(out=bv_f32, in_=bv[:].partition_broadcast(P))
        nc.vector.tensor_copy(bv_sb, bv_f32)

        # ---- persistent big tensors ----
        BT_sb = persist.tile([P, DC, S], BF16, tag="BT")
        V_sb = persist.tile([P, SB, D], BF16, tag="V")
        xT_sb = persist.tile([P, DC, S], BF16, tag="xT")
        Wv_sb = persist.tile([P, DC, D], BF16, tag="Wv")
        A_sb = persist.tile([P, DC, D], BF16, tag="A")
        WqT_sb = persist.tile([P, DC, DC, P], BF16, tag="WqT")
        WkT_sb = persist.tile([P, DC, HDC, P], BF16, tag="WkT")  # one d'-half
        t16 = persist.tile([P, DC], BF16, tag="t16")
        w_sb = persist.tile([P, SB], F32, tag="w")

        # ---- persistent pools ----
        xfp = ctx.enter_context(tc.tile_pool(name="xf", bufs=2))
        x16p = ctx.enter_context(tc.tile_pool(name="x16", bufs=2))
        wstgp = ctx.enter_context(tc.tile_pool(name="wstg", bufs=3))
        wslp = ctx.enter_context(tc.tile_pool(name="wsl", bufs=4))
        wqkp = ctx.enter_context(tc.tile_pool(name="wqk", bufs=4))
        ptp = ctx.enter_context(tc.tile_pool(name="pt", bufs=1))
        otp = ctx.enter_context(tc.tile_pool(name="ot", bufs=2))
        lstp = ctx.enter_context(tc.tile_pool(name="lst", bufs=4))
        tpsp = ctx.enter_context(tc.tile_pool(name="tps", bufs=1, space="PSUM"))
        workp = ctx.enter_context(tc.tile_pool(name="work", bufs=3, space="PSUM"))
        opsp = ctx.enter_context(tc.tile_pool(name="ops", bufs=2, space="PSUM"))

        def emit_rep():
            xf_tiles = {}

            def emit_x_load(sb):
                xf = xfp.tile([P, D], F32, tag="xf")
                nc.sync.dma_start(out=xf, in_=x[sb * P:(sb + 1) * P, :])
                xf_tiles[sb] = xf

            wvr = Wv.rearrange("(c p) e -> p c e", p=P)

            def load_wv_half(h):
                for ci in range(DC):
                    wst = wstgp.tile([P, PG], F32, tag="wst")
                    nc.sync.dma_start(
                        out=wst, in_=wvr[:, ci, h * PG:(h + 1) * PG])
                    nc.scalar.activation(
                        Wv_sb[:, ci, h * PG:(h + 1) * PG], wst,
                        AFT.Identity, scale=1.0)

            def load_w_hslice(W, c, h):
                # e-slice c, d'-half h of Wq/Wk: [P, HDC, P] fp32 -> bf16
                wr = W.rearrange("(c p) e -> p c e", p=P)
                wst = wslp.tile([P, HDC, P], F32, tag="wsl")
                nc.sync.dma_start(
                    out=wst,
                    in_=wr[:, h * HDC:(h + 1) * HDC, c * P:(c + 1) * P])
                wt = wqkp.tile([P, HDC, P], BF16, tag="wqk")
                nc.scalar.activation(wt, wst, AFT.Identity, scale=1.0)
                return wt

            # single transpose-PSUM generation per rep (W and x phases
            # share it; slice hazards pipeline the drains)
            pst = tpsp.tile([P, DC, P], BF16, tag="pst")

            # ---- Merged phase 0 + stage 1 ----
            # One interleaved pipeline: x transposes and V groups are the
            # PE filler while Wq/Wk slices stream and get transposed
            # (tiny PE), then A, B^T, w. Every engine queue receives its
            # ops in data-arrival order.
            with nc.named_scope("stage1"):
                def emit_transpose(sb):
                    x16 = x16p.tile([P, D], BF16, tag="x16")
                    nc.vector.tensor_copy(x16, xf_tiles.pop(sb))
                    for g in range(2):
                        for c4 in range(4):
                            c = g * 4 + c4
                            nc.tensor.transpose(
                                pst[:, c, :], x16[:, c * P:(c + 1) * P],
                                ident)
                        nc.vector.tensor_copy(
                            xT_sb[:, g * 4:g * 4 + 4, sb * P:(sb + 1) * P],
                            pst[:, g * 4:g * 4 + 4, :])

                def emit_wkt(c, h):
                    wt = load_w_hslice(Wk, c, h)
                    for cd in range(HDC):
                        nc.tensor.transpose(
                            pst[:, cd, :], wt[:, cd, :], ident)
                    nc.vector.tensor_copy(WkT_sb[:, c], pst[:, 0:HDC, :])

                def emit_wqt(c):
                    for h2 in range(2):
                        wt = load_w_hslice(Wq, c, h2)
                        for cd in range(HDC):
                            nc.tensor.transpose(
                                pst[:, h2 * HDC + cd, :], wt[:, cd, :],
                                ident)
                        nc.vector.tensor_copy(
                            WqT_sb[:, c, h2 * HDC:(h2 + 1) * HDC, :],
                            pst[:, h2 * HDC:(h2 + 1) * HDC, :])

                def emit_a(cd, h):
                    ps = workp.tile([P, PG], F32, tag="wps")
                    for ce in range(DC):
                        nc.tensor.matmul(
                            ps, WqT_sb[:, ce, cd, :], WkT_sb[:, ce],
                            start=(ce == 0), stop=(ce == DC - 1))
                    nc.scalar.activation(
                        A_sb[:, cd, h * PG:(h + 1) * PG], ps,
                        AFT.Identity, scale=1.0)

                def emit_t(cd, h):
                    tp = workp.tile([P, PG], F32, tag="wps")
                    for ce in range(DC):
                        nc.tensor.matmul(
                            tp[:, 0:1], WkT_sb[:, ce, cd, :],
                            bq16[:, ce:ce + 1],
                            start=(ce == 0), stop=(ce == DC - 1))
                    nc.vector.tensor_copy(
                        t16[:, h * HDC + cd:h * HDC + cd + 1], tp[:, 0:1])

                def emit_v(sb, h):
                    ps = workp.tile([P, PG], F32, tag="wps")
                    for ci in range(DC):
                        nc.tensor.matmul(
                            ps, xT_sb[:, ci, sb * P:(sb + 1) * P],
                            Wv_sb[:, ci, h * PG:(h + 1) * PG],
                            start=(ci == 0), stop=(ci == DC - 1))
                    nc.vector.tensor_add(
                        V_sb[:, sb, h * PG:(h + 1) * PG], ps,
                        bv_sb[:, h * PG:(h + 1) * PG])

                def emit_bt(cp, g):
                    ps = workp.tile([P, PG], F32, tag="wps")
                    for cd in range(DC):
                        nc.tensor.matmul(
                            ps, A_sb[:, cd, cp * P:(cp + 1) * P],
                            xT_sb[:, cd, g * PG:(g + 1) * PG],
                            start=(cd == 0), stop=(cd == DC - 1))
                    nc.scalar.activation(
                        BT_sb[:, cp, g * PG:(g + 1) * PG], ps,
                        AFT.Identity, scale=1.0)

                def emit_w(sb):
                    wp = workp.tile([P, PG], F32, tag="wps")
                    for ci in range(DC):
                        nc.tensor.matmul(
                            wp[:, 0:1], xT_sb[:, ci, sb * P:(sb + 1) * P],
                            t16[:, ci:ci + 1],
                            start=(ci == 0), stop=(ci == DC - 1))
                    nc.scalar.activation(w_sb[:, sb:sb + 1], wp[:, 0:1],
                                         AFT.Identity, scale=scale)

                # Ring order: x + Wv first (T and V groups keep PE
                # saturated through the whole x stream), then the Wq/Wk
                # slices, whose transposes + A + B^T chains follow as
                # self-paced fillers.
                emit_x_load(0)
                emit_x_load(1)
                emit_transpose(0)
                emit_transpose(1)
                load_wv_half(0)
                emit_x_load(2)
                emit_x_load(3)
                emit_transpose(2)
                emit_transpose(3)
                load_wv_half(1)
                emit_x_load(4)
                emit_x_load(5)
                emit_transpose(4)
                emit_transpose(5)
                emit_v(0, 0)
                emit_v(0, 1)
                vq = [(sb, h) for sb in range(1, SB) for h in range(2)]
                for sb in range(6, SB):
                    emit_x_load(sb)
                    emit_transpose(sb)
                    for _ in range(2):
                        if vq and vq[0][0] < sb:
                            emit_v(*vq.pop(0))
                for c in range(DC):
                    emit_wkt(c, 0)
                    if c % 2 and vq:
                        emit_v(*vq.pop(0))
                for c in range(DC):
                    emit_wqt(c)
                    if c % 2 and vq:
                        emit_v(*vq.pop(0))
                for cd in range(DC):
                    emit_a(cd, 0)
                    if cd % 4 == 3 and vq:
                        emit_v(*vq.pop(0))
                for cd in range(HDC):
                    emit_t(cd, 0)
                for c in range(DC):
                    emit_wkt(c, 1)
                    if c % 2 and vq:
                        emit_v(*vq.pop(0))
                for cd in range(DC):
                    emit_a(cd, 1)
                    if vq:
                        emit_v(*vq.pop(0))
                for cd in range(HDC):
                    emit_t(cd, 1)
                while vq:
                    emit_v(*vq.pop(0))
                for g in range(S // PG):
                    for cp in range(DC):
                        emit_bt(cp, g)
                for sb in range(SB):
                    emit_w(sb)

            # ---- Stage 2 ----
            with nc.named_scope("stage2"):
                for qg in range(NQG):
                    PT = ptp.tile([P, SB, PG], BF16, tag="PT")
                    q_lo = qg * PG
                    for kb in range(SB):
                        ps = workp.tile([P, PG], F32, tag="wps")
                        for c in range(DC):
                            nc.tensor.matmul(
                                ps, xT_sb[:, c, kb * P:(kb + 1) * P],
                                BT_sb[:, c, q_lo:q_lo + PG],
                                start=(c == 0), stop=(c == DC - 1))
                        nc.scalar.activation(PT[:, kb, :], ps, AFT.Exp,
                                             bias=w_sb[:, kb:kb + 1],
                                             scale=scale)
                    for qb in range(PG // P):
                        q0 = q_lo + qb * P
                        ps_o = opsp.tile([P, D], F32, tag="ps_o")
                        lt = workp.tile([P, PG], F32, tag="wps")
                        ps_l = lt[:, 0:1]
                        for kb in range(SB):
                            pt_s = PT[:, kb, qb * P:(qb + 1) * P]
                            for h in range(D // PG):
                                nc.tensor.matmul(
                                    ps_o[:, h * PG:(h + 1) * PG], pt_s,
                                    V_sb[:, kb, h * PG:(h + 1) * PG],
                                    start=(kb == 0), stop=(kb == SB - 1))
                            nc.tensor.matmul(ps_l, pt_s, ones,
                                             start=(kb == 0),
                                             stop=(kb == SB - 1))
                        rl = lstp.tile([P, 1], F32, tag="rl")
                        nc.vector.reciprocal(rl, ps_l)
                        for h in range(D // PG):
                            sl = slice(h * PG, (h + 1) * PG)
                            o_t = otp.tile([P, PG], F32, tag="o_t")
                            nc.vector.tensor_scalar_mul(o_t, ps_o[:, sl], rl)
                            nc.scalar.dma_start(out=y[q0:q0 + P, sl], in_=o_t)

        for _rep in range(reps):
            emit_rep()

    nc.compile()
    return nc


_NC_CACHE = {}


def _get_nc():
    if "nc" not in _NC_CACHE:
        _NC_CACHE["nc"] = build_attention_nc(S=S, D=D)
    return _NC_CACHE["nc"]


def run(inputs, trace=False, **run_kwargs):
    """Shard over batch, run on cores 0..7, gather. Returns (y, BassKernelResults)."""
    x = np.ascontiguousarray(np.asarray(inputs["x"], dtype=np.float32))
    shared = {
        k: np.ascontiguousarray(np.asarray(inputs[k], dtype=np.float32))
        for k in ("Wq", "Wk", "Wv", "bq", "bk", "bv")
    }
    in_maps = [dict(shared, x=x[b]) for b in range(B)]
    nc = _get_nc()
    res = run_bass_kernel_spmd(nc, in_maps, core_ids=list(range(N_CORES)),
                               trace=trace, **run_kwargs)
    y = np.stack([res.results[b]["y"] for b in range(B)], axis=0)
    return y, res


def kernel(**inputs):
    y, _ = run(inputs, trace=False)
    return y


# revision 4
# speedup vs baseline: 813.3986x; 1.0025x over previous
"""Trainium2 Bass kernel for single-head attention (AutoCorrelationLayer), v4.

v4 = v3 (all-bf16, transposed scores, persistent SBUF/PSUM, cross-rep
prefetch) + score reassociation:
    scores = (xWq + bq)(xWk + bk)^T
           = x (Wq Wk^T) x^T  +  [const over keys]  +  (x Wk bq)^T 1
so with A := Wq Wk^T (D^3/2 MACs, 65.5K PE cycles) the K-projection
(131K cycles) disappears; the surviving bias term w := x Wk bq /sqrt(D)
rides the per-partition bias input of the exp activation (keys are the
partition dim in the transposed-scores layout), and the other terms
cancel under softmax shift-invariance. Net PE: ~887K cycles vs ~934K.

Pipeline per rep:
  Phase 0: Wq/Wk stream in 128-col e-slices (fp32->bf16 on ACT); PE
    transposes them into WqT (full) / WkT (d'-half at a time); A = WqT^T
    chunks @ WkT halves accumulates in PSUM, drains bf16. t := Wk^T-
    chunks^T @ bq-chunks (rank-1, N=1 matmuls) -> t16.
  Stage 1: x blocks: fp32 -> bf16 (DVE) -> PE transpose -> xT resident.
    B^T = A-chunks^T @ xT (e-major, replaces qT), per 512-col g-span,
    interleaved behind the transposes; V = xT-slices^T @ Wv likewise;
    w = xT-slices^T @ t16 (N=1 chain) with the 1/sqrt(D) scale folded
    into the ACT drain.
  Stage 2: per 512-col q-group: sT = xT-chunk^T @ B^T + exp with
    bias=w[key-slice] -> P^T; PV + ones-column row sums; reciprocal,
    scale, store. No max-subtraction (|logits| <= ~7).

Sharding: data-parallel over batch, one element per core, as before.
"""

from contextlib import ExitStack

import numpy as np

import concourse.bacc as bacc
import concourse.bass as bass
import concourse.mybir as mybir
import concourse.tile as tile
from concourse.bass_utils import run_bass_kernel_spmd
from concourse.masks import make_identity

F32 = mybir.dt.float32
BF16 = mybir.dt.bfloat16
AFT = mybir.ActivationFunctionType
P = 128

B, S, D = 8, 2048, 1024
N_CORES = 8


def build_attention_nc(S=2048, D=1024, reps=1):
    nc = bacc.Bacc(dynamic_dma_scratch_size=4096)
    DC = D // P      # d/e chunks (8)
    SB = S // P      # s blocks (16)
    PG = 512         # projection span / stage-2 q-group width
    NQG = S // PG    # stage-2 q groups (4)
    HDC = DC // 2    # d'-half chunk count (4)
    scale = 1.0 / float(D) ** 0.5

    x = nc.dram_tensor("x", [S, D], F32, kind="ExternalInput")
    Wq = nc.dram_tensor("Wq", [D, D], F32, kind="ExternalInput")
    Wk = nc.dram_tensor("Wk", [D, D], F32, kind="ExternalInput")
    Wv = nc.dram_tensor("Wv", [D, D], F32, kind="ExternalInput")
    bq = nc.dram_tensor("bq", [D], F32, kind="ExternalInput")
    bk = nc.dram_tensor("bk", [D], F32, kind="ExternalInput")
    bv = nc.dram_tensor("bv", [D], F32, kind="ExternalInput")
    y = nc.dram_tensor("y", [S, D], F32, kind="ExternalOutput")

    with tile.TileContext(nc) as tc, ExitStack() as ctx:
        persist = ctx.enter_context(tc.tile_pool(name="persist", bufs=1))
        ident32 = persist.tile([P, P], F32, tag="ident32")
        make_identity(nc, ident32)
        ident = persist.tile([P, P], BF16, tag="ident")
        nc.vector.tensor_copy(ident, ident32)
        ones = persist.tile([P, 1], BF16, tag="ones")
        nc.gpsimd.memset(ones, 1.0)
        bq_sb = persist.tile([P, DC], F32, tag="bq")
        bq16 = persist.tile([P, DC], BF16, tag="bq16")
        bv_sb = persist.tile([P, D], BF16, tag="bv")
        bv_f32 = persist.tile([P, D], F32, tag="bvf")
        nc.gpsimd.dma_start(out=bq_sb, in_=bq.rearrange("(c p) -> p c", p=P))
        nc.vector.tensor_copy(bq16, bq_sb)
        nc.gpsimd.dma_start(out=bv_f32, in_=bv[:].partition_broadcast(P))
        nc.vector.tensor_copy(bv_sb, bv_f32)

        # ---- persistent big tensors ----
        BT_sb = persist.tile([P, DC, S], BF16, tag="BT")
        V_sb = persist.tile([P, SB, D], BF16, tag="V")
        xT_sb = persist.tile([P, DC, S], BF16, tag="xT")
        Wv_sb = persist.tile([P, DC, D], BF16, tag="Wv")
        A_sb = persist.tile([P, DC, D], BF16, tag="A")
        WqT_sb = persist.tile([P, DC, DC, P], BF16, tag="WqT")
        WkT_sb = persist.tile([P, DC, HDC, P], BF16, tag="WkT")  # one d'-half
        t16 = persist.tile([P, DC], BF16, tag="t16")
        w_sb = persist.tile([P, SB], F32, tag="w")

        # ---- persistent pools ----
        xfp = ctx.enter_context(tc.tile_pool(name="xf", bufs=2))
        x16p = ctx.enter_context(tc.tile_pool(name="x16", bufs=2))
        wstgp = ctx.enter_context(tc.tile_pool(name="wstg", bufs=4))
        wslp = ctx.enter_context(tc.tile_pool(name="wsl", bufs=6))
        wqkp = ctx.enter_context(tc.tile_pool(name="wqk", bufs=6))
        ptp = ctx.enter_context(tc.tile_pool(name="pt", bufs=1))
        otp = ctx.enter_context(tc.tile_pool(name="ot", bufs=2))
        lstp = ctx.enter_context(tc.tile_pool(name="lst", bufs=4))
        tpsp = ctx.enter_context(tc.tile_pool(name="tps", bufs=1, space="PSUM"))
        workp = ctx.enter_context(tc.tile_pool(name="work", bufs=3, space="PSUM"))
        opsp = ctx.enter_context(tc.tile_pool(name="ops", bufs=2, space="PSUM"))

        def emit_rep():
            xf_tiles = {}

            def emit_x_load(sb):
                xf = xfp.tile([P, D], F32, tag="xf")
                nc.sync.dma_start(out=xf, in_=x[sb * P:(sb + 1) * P, :])
                xf_tiles[sb] = xf

            wvr = Wv.rearrange("(c p) e -> p c e", p=P)

            def load_wv_half(h):
                for ci in range(DC):
                    wst = wstgp.tile([P, PG], F32, tag="wst")
                    nc.sync.dma_start(
                        out=wst, in_=wvr[:, ci, h * PG:(h + 1) * PG])
                    nc.scalar.activation(
                        Wv_sb[:, ci, h * PG:(h + 1) * PG], wst,
                        AFT.Identity, scale=1.0)

            def load_w_hslice(W, c, h):
                # e-slice c, d'-half h of Wq/Wk: [P, HDC, P] fp32 -> bf16
                wr = W.rearrange("(c p) e -> p c e", p=P)
                wst = wslp.tile([P, HDC, P], F32, tag="wsl")
                nc.sync.dma_start(
                    out=wst,
                    in_=wr[:, h * HDC:(h + 1) * HDC, c * P:(c + 1) * P])
                wt = wqkp.tile([P, HDC, P], BF16, tag="wqk")
                nc.scalar.activation(wt, wst, AFT.Identity, scale=1.0)
                return wt

            # single transpose-PSUM generation per rep (W and x phases
            # share it; slice hazards pipeline the drains)
            pst = tpsp.tile([P, DC, P], BF16, tag="pst")

            # ---- Merged phase 0 + stage 1 ----
            # One interleaved pipeline: x transposes and V groups are the
            # PE filler while Wq/Wk slices stream and get transposed
            # (tiny PE), then A, B^T, w. Every engine queue receives its
            # ops in data-arrival order.
            with nc.named_scope("stage1"):
                def emit_transpose(sb):
                    x16 = x16p.tile([P, D], BF16, tag="x16")
                    nc.vector.tensor_copy(x16, xf_tiles.pop(sb))
                    for g in range(2):
                        for c4 in range(4):
                            c = g * 4 + c4
                            nc.tensor.transpose(
                                pst[:, c, :], x16[:, c * P:(c + 1) * P],
                                ident)
                        nc.vector.tensor_copy(
                            xT_sb[:, g * 4:g * 4 + 4, sb * P:(sb + 1) * P],
                            pst[:, g * 4:g * 4 + 4, :])

                def emit_wkt(c, h):
                    wt = load_w_hslice(Wk, c, h)
                    for cd in range(HDC):
                        nc.tensor.transpose(
                            pst[:, cd, :], wt[:, cd, :], ident)
                    nc.vector.tensor_copy(WkT_sb[:, c], pst[:, 0:HDC, :])

                def emit_wqt(c):
                    for h2 in range(2):
                        wt = load_w_hslice(Wq, c, h2)
                        for cd in range(HDC):
                            nc.tensor.transpose(
                                pst[:, h2 * HDC + cd, :], wt[:, cd, :],
                                ident)
                        nc.vector.tensor_copy(
                            WqT_sb[:, c, h2 * HDC:(h2 + 1) * HDC, :],
                            pst[:, h2 * HDC:(h2 + 1) * HDC, :])

                def emit_a(cd, h):
                    ps = workp.tile([P, PG], F32, tag="wps")
                    for ce in range(DC):
                        nc.tensor.matmul(
                            ps, WqT_sb[:, ce, cd, :], WkT_sb[:, ce],
                            start=(ce == 0), stop=(ce == DC - 1))
                    nc.scalar.activation(
                        A_sb[:, cd, h * PG:(h + 1) * PG], ps,
                        AFT.Identity, scale=1.0)

                def emit_t(cd, h):
                    tp = workp.tile([P, PG], F32, tag="wps")
                    for ce in range(DC):
                        nc.tensor.matmul(
                            tp[:, 0:1], WkT_sb[:, ce, cd, :],
                            bq16[:, ce:ce + 1],
                            start=(ce == 0), stop=(ce == DC - 1))
                    nc.vector.tensor_copy(
                        t16[:, h * HDC + cd:h * HDC + cd + 1], tp[:, 0:1])

                def emit_v(sb, h):
                    ps = workp.tile([P, PG], F32, tag="wps")
                    for ci in range(DC):
                        nc.tensor.matmul(
                            ps, xT_sb[:, ci, sb * P:(sb + 1) * P],
                            Wv_sb[:, ci, h * PG:(h + 1) * PG],
                            start=(ci == 0), stop=(ci == DC - 1))
                    nc.vector.tensor_add(
                        V_sb[:, sb, h * PG:(h + 1) * PG], ps,
                        bv_sb[:, h * PG:(h + 1) * PG])

                def emit_bt(cp, g):
                    ps = workp.tile([P, PG], F32, tag="wps")
                    for cd in range(DC):
                        nc.tensor.matmul(
                            ps, A_sb[:, cd, cp * P:(cp + 1) * P],
                            xT_sb[:, cd, g * PG:(g + 1) * PG],
                            start=(cd == 0), stop=(cd == DC - 1))
                    nc.scalar.activation(
                        BT_sb[:, cp, g * PG:(g + 1) * PG], ps,
                        AFT.Identity, scale=1.0)

                def emit_w(sb):
                    wp = workp.tile([P, PG], F32, tag="wps")
                    for ci in range(DC):
                        nc.tensor.matmul(
                            wp[:, 0:1], xT_sb[:, ci, sb * P:(sb + 1) * P],
                            t16[:, ci:ci + 1],
                            start=(ci == 0), stop=(ci == DC - 1))
                    nc.scalar.activation(w_sb[:, sb:sb + 1], wp[:, 0:1],
                                         AFT.Identity, scale=scale)

                # Ring order: x + Wv first (T and V groups keep PE
                # saturated through the whole x stream), then the Wq/Wk
                # slices, whose transposes + A + B^T chains follow as
                # self-paced fillers.
                emit_x_load(0)
                emit_x_load(1)
                emit_transpose(0)
                emit_transpose(1)
                load_wv_half(0)
                emit_x_load(2)
                emit_x_load(3)
                emit_transpose(2)
                emit_transpose(3)
                load_wv_half(1)
                emit_x_load(4)
                emit_x_load(5)
                emit_transpose(4)
                emit_transpose(5)
                emit_v(0, 0)
                emit_v(0, 1)
                vq = [(sb, h) for sb in range(1, SB) for h in range(2)]
                for sb in range(6, SB):
                    emit_x_load(sb)
                    emit_transpose(sb)
                    for _ in range(2):
                        if vq and vq[0][0] < sb:
                            emit_v(*vq.pop(0))
                for c in range(DC):
                    emit_wkt(c, 0)
                    if c % 2 and vq:
                        emit_v(*vq.pop(0))
                for c in range(DC):
                    emit_wqt(c)
                    if c % 2 and vq:
                        emit_v(*vq.pop(0))
                for cd in range(DC):
                    emit_a(cd, 0)
                    if cd % 4 == 3 and vq:
                        emit_v(*vq.pop(0))
                for cd in range(HDC):
                    emit_t(cd, 0)
                for c in range(DC):
                    emit_wkt(c, 1)
                    if c % 2 and vq:
                        emit_v(*vq.pop(0))
                for cd in range(DC):
                    emit_a(cd, 1)
                    if vq:
                        emit_v(*vq.pop(0))
                for cd in range(HDC):
                    emit_t(cd, 1)
                while vq:
                    emit_v(*vq.pop(0))
                for g in range(S // PG):
                    for cp in range(DC):
                        emit_bt(cp, g)
                for sb in range(SB):
                    emit_w(sb)

            # ---- Stage 2 ----
            with nc.named_scope("stage2"):
                for qg in range(NQG):
                    PT = ptp.tile([P, SB, PG], BF16, tag="PT")
                    q_lo = qg * PG
                    for kb in range(SB):
                        ps = workp.tile([P, PG], F32, tag="wps")
                        for c in range(DC):
                            nc.tensor.matmul(
                                ps, xT_sb[:, c, kb * P:(kb + 1) * P],
                                BT_sb[:, c, q_lo:q_lo + PG],
                                start=(c == 0), stop=(c == DC - 1))
                        nc.scalar.activation(PT[:, kb, :], ps, AFT.Exp,
                                             bias=w_sb[:, kb:kb + 1],
                                             scale=scale)
                    for qb in range(PG // P):
                        q0 = q_lo + qb * P
                        ps_o = opsp.tile([P, D], F32, tag="ps_o")
                        lt = workp.tile([P, PG], F32, tag="wps")
                        ps_l = lt[:, 0:1]
                        for kb in range(SB):
                            pt_s = PT[:, kb, qb * P:(qb + 1) * P]
                            for h in range(D // PG):
                                nc.tensor.matmul(
                                    ps_o[:, h * PG:(h + 1) * PG], pt_s,
                                    V_sb[:, kb, h * PG:(h + 1) * PG],
                                    start=(kb == 0), stop=(kb == SB - 1))
                            nc.tensor.matmul(ps_l, pt_s, ones,
                                             start=(kb == 0),
                                             stop=(kb == SB - 1))
                        rl = lstp.tile([P, 1], F32, tag="rl")
                        nc.vector.reciprocal(rl, ps_l)
                        for h in range(D // PG):
                            sl = slice(h * PG, (h + 1) * PG)
                            o_t = otp.tile([P, PG], F32, tag="o_t")
                            nc.vector.tensor_scalar_mul(o_t, ps_o[:, sl], rl)
                            nc.scalar.dma_start(out=y[q0:q0 + P, sl], in_=o_t)

        for _rep in range(reps):
            emit_rep()

    nc.compile()
    return nc


_NC_CACHE = {}


def _get_nc():
    if "nc" not in _NC_CACHE:
        _NC_CACHE["nc"] = build_attention_nc(S=S, D=D)
    return _NC_CACHE["nc"]


def run(inputs, trace=False, **run_kwargs):
    """Shard over batch, run on cores 0..7, gather. Returns (y, BassKernelResults)."""
    x = np.ascontiguousarray(np.asarray(inputs["x"], dtype=np.float32))
    shared = {
        k: np.ascontiguousarray(np.asarray(inputs[k], dtype=np.float32))
        for k in ("Wq", "Wk", "Wv", "bq", "bk", "bv")
    }
    in_maps = [dict(shared, x=x[b]) for b in range(B)]
    nc = _get_nc()
    res = run_bass_kernel_spmd(nc, in_maps, core_ids=list(range(N_CORES)),
                               trace=trace, **run_kwargs)
    y = np.stack([res.results[b]["y"] for b in range(B)], axis=0)
    return y, res


def kernel(**inputs):
    y, _ = run(inputs, trace=False)
    return y


# revision 5
# speedup vs baseline: 814.7512x; 1.0017x over previous
"""Trainium2 Bass kernel for single-head attention (AutoCorrelationLayer), v4.

v4 = v3 (all-bf16, transposed scores, persistent SBUF/PSUM, cross-rep
prefetch) + score reassociation:
    scores = (xWq + bq)(xWk + bk)^T
           = x (Wq Wk^T) x^T  +  [const over keys]  +  (x Wk bq)^T 1
so with A := Wq Wk^T (D^3/2 MACs, 65.5K PE cycles) the K-projection
(131K cycles) disappears; the surviving bias term w := x Wk bq /sqrt(D)
rides the per-partition bias input of the exp activation (keys are the
partition dim in the transposed-scores layout), and the other terms
cancel under softmax shift-invariance. Net PE: ~887K cycles vs ~934K.

Pipeline per rep:
  Phase 0: Wq/Wk stream in 128-col e-slices (fp32->bf16 on ACT); PE
    transposes them into WqT (full) / WkT (d'-half at a time); A = WqT^T
    chunks @ WkT halves accumulates in PSUM, drains bf16. t := Wk^T-
    chunks^T @ bq-chunks (rank-1, N=1 matmuls) -> t16.
  Stage 1: x blocks: fp32 -> bf16 (DVE) -> PE transpose -> xT resident.
    B^T = A-chunks^T @ xT (e-major, replaces qT), per 512-col g-span,
    interleaved behind the transposes; V = xT-slices^T @ Wv likewise;
    w = xT-slices^T @ t16 (N=1 chain) with the 1/sqrt(D) scale folded
    into the ACT drain.
  Stage 2: per 512-col q-group: sT = xT-chunk^T @ B^T + exp with
    bias=w[key-slice] -> P^T; PV + ones-column row sums; reciprocal,
    scale, store. No max-subtraction (|logits| <= ~7).

Sharding: data-parallel over batch, one element per core, as before.
"""

from contextlib import ExitStack

import numpy as np

import concourse.bacc as bacc
import concourse.bass as bass
import concourse.mybir as mybir
import concourse.tile as tile
from concourse.bass_utils import run_bass_kernel_spmd
from concourse.masks import make_identity

F32 = mybir.dt.float32
BF16 = mybir.dt.bfloat16
AFT = mybir.ActivationFunctionType
P = 128

B, S, D = 8, 2048, 1024
N_CORES = 8


def build_attention_nc(S=2048, D=1024, reps=1):
    nc = bacc.Bacc(dynamic_dma_scratch_size=4096)
    DC = D // P      # d/e chunks (8)
    SB = S // P      # s blocks (16)
    PG = 512         # projection span / stage-2 q-group width
    NQG = S // PG    # stage-2 q groups (4)
    HDC = DC // 2    # d'-half chunk count (4)
    scale = 1.0 / float(D) ** 0.5

    x = nc.dram_tensor("x", [S, D], F32, kind="ExternalInput")
    Wq = nc.dram_tensor("Wq", [D, D], F32, kind="ExternalInput")
    Wk = nc.dram_tensor("Wk", [D, D], F32, kind="ExternalInput")
    Wv = nc.dram_tensor("Wv", [D, D], F32, kind="ExternalInput")
    bq = nc.dram_tensor("bq", [D], F32, kind="ExternalInput")
    bk = nc.dram_tensor("bk", [D], F32, kind="ExternalInput")
    bv = nc.dram_tensor("bv", [D], F32, kind="ExternalInput")
    y = nc.dram_tensor("y", [S, D], F32, kind="ExternalOutput")

    with tile.TileContext(nc) as tc, ExitStack() as ctx:
        persist = ctx.enter_context(tc.tile_pool(name="persist", bufs=1))
        ident32 = persist.tile([P, P], F32, tag="ident32")
        make_identity(nc, ident32)
        ident = persist.tile([P, P], BF16, tag="ident")
        nc.vector.tensor_copy(ident, ident32)
        ones = persist.tile([P, 1], BF16, tag="ones")
        nc.gpsimd.memset(ones, 1.0)
        bq_sb = persist.tile([P, DC], F32, tag="bq")
        bq16 = persist.tile([P, DC], BF16, tag="bq16")
        bv_sb = persist.tile([P, D], BF16, tag="bv")
        bv_f32 = persist.tile([P, D], F32, tag="bvf")
        nc.gpsimd.dma_start(out=bq_sb, in_=bq.rearrange("(c p) -> p c", p=P))
        nc.vector.tensor_copy(bq16, bq_sb)
        nc.gpsimd.dma_start(out=bv_f32, in_=bv[:].partition_broadcast(P))
        nc.vector.tensor_copy(bv_sb, bv_f32)

        # ---- persistent big tensors ----
        BT_sb = persist.tile([P, DC, S], BF16, tag="BT")
        V_sb = persist.tile([P, SB, D], BF16, tag="V")
        xT_sb = persist.tile([P, DC, S], BF16, tag="xT")
        Wv_sb = persist.tile([P, DC, D], BF16, tag="Wv")
        A_sb = persist.tile([P, DC, D], BF16, tag="A")
        WqT_sb = persist.tile([P, DC, DC, P], BF16, tag="WqT")
        WkT_sb = persist.tile([P, DC, HDC, P], BF16, tag="WkT")  # one d'-half
        t16 = persist.tile([P, DC], BF16, tag="t16")
        w_sb = persist.tile([P, SB], F32, tag="w")

        # ---- persistent pools ----
        xfp = ctx.enter_context(tc.tile_pool(name="xf", bufs=2))
        x16p = ctx.enter_context(tc.tile_pool(name="x16", bufs=2))
        wstgp = ctx.enter_context(tc.tile_pool(name="wstg", bufs=4))
        wslp = ctx.enter_context(tc.tile_pool(name="wsl", bufs=6))
        wqkp = ctx.enter_context(tc.tile_pool(name="wqk", bufs=6))
        ptp = ctx.enter_context(tc.tile_pool(name="pt", bufs=1))
        otp = ctx.enter_context(tc.tile_pool(name="ot", bufs=2))
        lstp = ctx.enter_context(tc.tile_pool(name="lst", bufs=4))
        tpsp = ctx.enter_context(tc.tile_pool(name="tps", bufs=1, space="PSUM"))
        workp = ctx.enter_context(tc.tile_pool(name="work", bufs=3, space="PSUM"))
        opsp = ctx.enter_context(tc.tile_pool(name="ops", bufs=2, space="PSUM"))

        def emit_rep():
            xf_tiles = {}

            def emit_x_load(sb):
                xf = xfp.tile([P, D], F32, tag="xf")
                nc.sync.dma_start(out=xf, in_=x[sb * P:(sb + 1) * P, :])
                xf_tiles[sb] = xf

            wvr = Wv.rearrange("(c p) e -> p c e", p=P)

            def load_wv_half(h):
                for ci in range(DC):
                    wst = wstgp.tile([P, PG], F32, tag="wst")
                    nc.sync.dma_start(
                        out=wst, in_=wvr[:, ci, h * PG:(h + 1) * PG])
                    nc.scalar.activation(
                        Wv_sb[:, ci, h * PG:(h + 1) * PG], wst,
                        AFT.Identity, scale=1.0)

            def load_w_hslice(W, c, h):
                # e-slice c, d'-half h of Wq/Wk: [P, HDC, P] fp32 -> bf16
                wr = W.rearrange("(c p) e -> p c e", p=P)
                wst = wslp.tile([P, HDC, P], F32, tag="wsl")
                # ACT ring: these have no data waits, so they prefetch in
                # parallel with the sync ring's x stream instead of
                # queueing behind it
                nc.scalar.dma_start(
                    out=wst,
                    in_=wr[:, h * HDC:(h + 1) * HDC, c * P:(c + 1) * P])
                wt = wqkp.tile([P, HDC, P], BF16, tag="wqk")
                nc.scalar.activation(wt, wst, AFT.Identity, scale=1.0)
                return wt

            # single transpose-PSUM generation per rep (W and x phases
            # share it; slice hazards pipeline the drains)
            pst = tpsp.tile([P, DC, P], BF16, tag="pst")

            # ---- Merged phase 0 + stage 1 ----
            # One interleaved pipeline: x transposes and V groups are the
            # PE filler while Wq/Wk slices stream and get transposed
            # (tiny PE), then A, B^T, w. Every engine queue receives its
            # ops in data-arrival order.
            with nc.named_scope("stage1"):
                def emit_transpose(sb):
                    x16 = x16p.tile([P, D], BF16, tag="x16")
                    nc.vector.tensor_copy(x16, xf_tiles.pop(sb))
                    for g in range(2):
                        for c4 in range(4):
                            c = g * 4 + c4
                            nc.tensor.transpose(
                                pst[:, c, :], x16[:, c * P:(c + 1) * P],
                                ident)
                        nc.vector.tensor_copy(
                            xT_sb[:, g * 4:g * 4 + 4, sb * P:(sb + 1) * P],
                            pst[:, g * 4:g * 4 + 4, :])

                def emit_wkt(c, h):
                    wt = load_w_hslice(Wk, c, h)
                    for cd in range(HDC):
                        nc.tensor.transpose(
                            pst[:, cd, :], wt[:, cd, :], ident)
                    nc.vector.tensor_copy(WkT_sb[:, c], pst[:, 0:HDC, :])

                def emit_wqt(c):
                    for h2 in range(2):
                        wt = load_w_hslice(Wq, c, h2)
                        for cd in range(HDC):
                            nc.tensor.transpose(
                                pst[:, h2 * HDC + cd, :], wt[:, cd, :],
                                ident)
                        nc.vector.tensor_copy(
                            WqT_sb[:, c, h2 * HDC:(h2 + 1) * HDC, :],
                            pst[:, h2 * HDC:(h2 + 1) * HDC, :])

                def emit_a(cd, h):
                    ps = workp.tile([P, PG], F32, tag="wps")
                    for ce in range(DC):
                        nc.tensor.matmul(
                            ps, WqT_sb[:, ce, cd, :], WkT_sb[:, ce],
                            start=(ce == 0), stop=(ce == DC - 1))
                    nc.scalar.activation(
                        A_sb[:, cd, h * PG:(h + 1) * PG], ps,
                        AFT.Identity, scale=1.0)

                def emit_t(cd, h):
                    tp = workp.tile([P, PG], F32, tag="wps")
                    for ce in range(DC):
                        nc.tensor.matmul(
                            tp[:, 0:1], WkT_sb[:, ce, cd, :],
                            bq16[:, ce:ce + 1],
                            start=(ce == 0), stop=(ce == DC - 1))
                    nc.vector.tensor_copy(
                        t16[:, h * HDC + cd:h * HDC + cd + 1], tp[:, 0:1])

                def emit_v(sb, h):
                    ps = workp.tile([P, PG], F32, tag="wps")
                    for ci in range(DC):
                        nc.tensor.matmul(
                            ps, xT_sb[:, ci, sb * P:(sb + 1) * P],
                            Wv_sb[:, ci, h * PG:(h + 1) * PG],
                            start=(ci == 0), stop=(ci == DC - 1))
                    nc.vector.tensor_add(
                        V_sb[:, sb, h * PG:(h + 1) * PG], ps,
                        bv_sb[:, h * PG:(h + 1) * PG])

                def emit_bt(cp, g):
                    ps = workp.tile([P, PG], F32, tag="wps")
                    for cd in range(DC):
                        nc.tensor.matmul(
                            ps, A_sb[:, cd, cp * P:(cp + 1) * P],
                            xT_sb[:, cd, g * PG:(g + 1) * PG],
                            start=(cd == 0), stop=(cd == DC - 1))
                    nc.scalar.activation(
                        BT_sb[:, cp, g * PG:(g + 1) * PG], ps,
                        AFT.Identity, scale=1.0)

                def emit_w(sb):
                    wp = workp.tile([P, PG], F32, tag="wps")
                    for ci in range(DC):
                        nc.tensor.matmul(
                            wp[:, 0:1], xT_sb[:, ci, sb * P:(sb + 1) * P],
                            t16[:, ci:ci + 1],
                            start=(ci == 0), stop=(ci == DC - 1))
                    nc.scalar.activation(w_sb[:, sb:sb + 1], wp[:, 0:1],
                                         AFT.Identity, scale=scale)

                # Ring order: x + Wv first (T and V groups keep PE
                # saturated through the whole x stream), then the Wq/Wk
                # slices, whose transposes + A + B^T chains follow as
                # self-paced fillers.
                emit_x_load(0)
                emit_x_load(1)
                emit_transpose(0)
                emit_transpose(1)
                load_wv_half(0)
                emit_x_load(2)
                emit_x_load(3)
                emit_transpose(2)
                emit_transpose(3)
                load_wv_half(1)
                emit_x_load(4)
                emit_x_load(5)
                emit_transpose(4)
                emit_transpose(5)
                emit_v(0, 0)
                emit_v(0, 1)
                vq = [(sb, h) for sb in range(1, SB) for h in range(2)]
                for sb in range(6, SB):
                    emit_x_load(sb)
                    emit_transpose(sb)
                    for _ in range(2):
                        if vq and vq[0][0] < sb:
                            emit_v(*vq.pop(0))
                for c in range(DC):
                    emit_wkt(c, 0)
                    if c % 2 and vq:
                        emit_v(*vq.pop(0))
                for c in range(DC):
                    emit_wqt(c)
                    if c % 2 and vq:
                        emit_v(*vq.pop(0))
                for cd in range(DC):
                    emit_a(cd, 0)
                    if cd % 4 == 3 and vq:
                        emit_v(*vq.pop(0))
                for cd in range(HDC):
                    emit_t(cd, 0)
                for c in range(DC):
                    emit_wkt(c, 1)
                    if c % 2 and vq:
                        emit_v(*vq.pop(0))
                for cd in range(DC):
                    emit_a(cd, 1)
                    if vq:
                        emit_v(*vq.pop(0))
                for cd in range(HDC):
                    emit_t(cd, 1)
                while vq:
                    emit_v(*vq.pop(0))
                for g in range(S // PG):
                    for cp in range(DC):
                        emit_bt(cp, g)
                for sb in range(SB):
                    emit_w(sb)

            # ---- Stage 2 ----
            with nc.named_scope("stage2"):
                for qg in range(NQG):
                    PT = ptp.tile([P, SB, PG], BF16, tag="PT")
                    q_lo = qg * PG
                    for kb in range(SB):
                        ps = workp.tile([P, PG], F32, tag="wps")
                        for c in range(DC):
                            nc.tensor.matmul(
                                ps, xT_sb[:, c, kb * P:(kb + 1) * P],
                                BT_sb[:, c, q_lo:q_lo + PG],
                                start=(c == 0), stop=(c == DC - 1))
                        nc.scalar.activation(PT[:, kb, :], ps, AFT.Exp,
                                             bias=w_sb[:, kb:kb + 1],
                                             scale=scale)
                    for qb in range(PG // P):
                        q0 = q_lo + qb * P
                        ps_o = opsp.tile([P, D], F32, tag="ps_o")
                        lt = workp.tile([P, PG], F32, tag="wps")
                        ps_l = lt[:, 0:1]
                        for kb in range(SB):
                            pt_s = PT[:, kb, qb * P:(qb + 1) * P]
                            for h in range(D // PG):
                                nc.tensor.matmul(
                                    ps_o[:, h * PG:(h + 1) * PG], pt_s,
                                    V_sb[:, kb, h * PG:(h + 1) * PG],
                                    start=(kb == 0), stop=(kb == SB - 1))
                            nc.tensor.matmul(ps_l, pt_s, ones,
                                             start=(kb == 0),
                                             stop=(kb == SB - 1))
                        rl = lstp.tile([P, 1], F32, tag="rl")
                        nc.vector.reciprocal(rl, ps_l)
                        for h in range(D // PG):
                            sl = slice(h * PG, (h + 1) * PG)
                            o_t = otp.tile([P, PG], F32, tag="o_t")
                            nc.vector.tensor_scalar_mul(o_t, ps_o[:, sl], rl)
                            nc.scalar.dma_start(out=y[q0:q0 + P, sl], in_=o_t)

        for _rep in range(reps):
            emit_rep()

    nc.compile()
    return nc


_NC_CACHE = {}


def _get_nc():
    if "nc" not in _NC_CACHE:
        _NC_CACHE["nc"] = build_attention_nc(S=S, D=D)
    return _NC_CACHE["nc"]


def run(inputs, trace=False, **run_kwargs):
    """Shard over batch, run on cores 0..7, gather. Returns (y, BassKernelResults)."""
    x = np.ascontiguousarray(np.asarray(inputs["x"], dtype=np.float32))
    shared = {
        k: np.ascontiguousarray(np.asarray(inputs[k], dtype=np.float32))
        for k in ("Wq", "Wk", "Wv", "bq", "bk", "bv")
    }
    in_maps = [dict(shared, x=x[b]) for b in range(B)]
    nc = _get_nc()
    res = run_bass_kernel_spmd(nc, in_maps, core_ids=list(range(N_CORES)),
                               trace=trace, **run_kwargs)
    y = np.stack([res.results[b]["y"] for b in range(B)], axis=0)
    return y, res


def kernel(**inputs):
    y, _ = run(inputs, trace=False)
    return y


# revision 6
# speedup vs baseline: 819.5212x; 1.0059x over previous
"""Trainium2 Bass kernel for single-head attention (AutoCorrelationLayer), v4.

v4 = v3 (all-bf16, transposed scores, persistent SBUF/PSUM, cross-rep
prefetch) + score reassociation:
    scores = (xWq + bq)(xWk + bk)^T
           = x (Wq Wk^T) x^T  +  [const over keys]  +  (x Wk bq)^T 1
so with A := Wq Wk^T (D^3/2 MACs, 65.5K PE cycles) the K-projection
(131K cycles) disappears; the surviving bias term w := x Wk bq /sqrt(D)
rides the per-partition bias input of the exp activation (keys are the
partition dim in the transposed-scores layout), and the other terms
cancel under softmax shift-invariance. Net PE: ~887K cycles vs ~934K.

Pipeline per rep:
  Phase 0: Wq/Wk stream in 128-col e-slices (fp32->bf16 on ACT); PE
    transposes them into WqT (full) / WkT (d'-half at a time); A = WqT^T
    chunks @ WkT halves accumulates in PSUM, drains bf16. t := Wk^T-
    chunks^T @ bq-chunks (rank-1, N=1 matmuls) -> t16.
  Stage 1: x blocks: fp32 -> bf16 (DVE) -> PE transpose -> xT resident.
    B^T = A-chunks^T @ xT (e-major, replaces qT), per 512-col g-span,
    interleaved behind the transposes; V = xT-slices^T @ Wv likewise;
    w = xT-slices^T @ t16 (N=1 chain) with the 1/sqrt(D) scale folded
    into the ACT drain.
  Stage 2: per 512-col q-group: sT = xT-chunk^T @ B^T + exp with
    bias=w[key-slice] -> P^T; PV + ones-column row sums; reciprocal,
    scale, store. No max-subtraction (|logits| <= ~7).

Sharding: data-parallel over batch, one element per core, as before.
"""

from contextlib import ExitStack

import numpy as np

import concourse.bacc as bacc
import concourse.bass as bass
import concourse.mybir as mybir
import concourse.tile as tile
from concourse.bass_utils import run_bass_kernel_spmd
from concourse.masks import make_identity

F32 = mybir.dt.float32
BF16 = mybir.dt.bfloat16
AFT = mybir.ActivationFunctionType
P = 128

B, S, D = 8, 2048, 1024
N_CORES = 8


def build_attention_nc(S=2048, D=1024, reps=1):
    nc = bacc.Bacc(dynamic_dma_scratch_size=4096)
    DC = D // P      # d/e chunks (8)
    SB = S // P      # s blocks (16)
    PG = 512         # projection span / stage-2 q-group width
    NQG = S // PG    # stage-2 q groups (4)
    HDC = DC // 2    # d'-half chunk count (4)
    scale = 1.0 / float(D) ** 0.5

    x = nc.dram_tensor("x", [S, D], F32, kind="ExternalInput")
    Wq = nc.dram_tensor("Wq", [D, D], F32, kind="ExternalInput")
    Wk = nc.dram_tensor("Wk", [D, D], F32, kind="ExternalInput")
    Wv = nc.dram_tensor("Wv", [D, D], F32, kind="ExternalInput")
    bq = nc.dram_tensor("bq", [D], F32, kind="ExternalInput")
    bk = nc.dram_tensor("bk", [D], F32, kind="ExternalInput")
    bv = nc.dram_tensor("bv", [D], F32, kind="ExternalInput")
    y = nc.dram_tensor("y", [S, D], F32, kind="ExternalOutput")

    with tile.TileContext(nc) as tc, ExitStack() as ctx:
        persist = ctx.enter_context(tc.tile_pool(name="persist", bufs=1))
        ident32 = persist.tile([P, P], F32, tag="ident32")
        make_identity(nc, ident32)
        ident = persist.tile([P, P], BF16, tag="ident")
        nc.vector.tensor_copy(ident, ident32)
        ones = persist.tile([P, 1], BF16, tag="ones")
        nc.gpsimd.memset(ones, 1.0)
        bq_sb = persist.tile([P, DC], F32, tag="bq")
        bq16 = persist.tile([P, DC], BF16, tag="bq16")
        bv_sb = persist.tile([P, D], BF16, tag="bv")
        bv_f32 = persist.tile([P, D], F32, tag="bvf")
        nc.gpsimd.dma_start(out=bq_sb, in_=bq.rearrange("(c p) -> p c", p=P))
        nc.vector.tensor_copy(bq16, bq_sb)
        nc.gpsimd.dma_start(out=bv_f32, in_=bv[:].partition_broadcast(P))
        nc.vector.tensor_copy(bv_sb, bv_f32)

        # ---- persistent big tensors ----
        BT_sb = persist.tile([P, DC, S], BF16, tag="BT")
        V_sb = persist.tile([P, SB, D], BF16, tag="V")
        xT_sb = persist.tile([P, DC, S], BF16, tag="xT")
        Wv_sb = persist.tile([P, DC, D], BF16, tag="Wv")
        A_sb = persist.tile([P, DC, D], BF16, tag="A")
        WqT_sb = persist.tile([P, DC, DC, P], BF16, tag="WqT")
        WkT_sb = persist.tile([P, DC, HDC, P], BF16, tag="WkT")  # one d'-half
        t16 = persist.tile([P, DC], BF16, tag="t16")
        w_sb = persist.tile([P, SB], F32, tag="w")

        # ---- persistent pools ----
        xfp = ctx.enter_context(tc.tile_pool(name="xf", bufs=2))
        x16p = ctx.enter_context(tc.tile_pool(name="x16", bufs=2))
        wstgp = ctx.enter_context(tc.tile_pool(name="wstg", bufs=4))
        wslp = ctx.enter_context(tc.tile_pool(name="wsl", bufs=6))
        wqkp = ctx.enter_context(tc.tile_pool(name="wqk", bufs=6))
        ptp = ctx.enter_context(tc.tile_pool(name="pt", bufs=1))
        otp = ctx.enter_context(tc.tile_pool(name="ot", bufs=2))
        lstp = ctx.enter_context(tc.tile_pool(name="lst", bufs=4))
        tpsp = ctx.enter_context(tc.tile_pool(name="tps", bufs=1, space="PSUM"))
        workp = ctx.enter_context(tc.tile_pool(name="work", bufs=3, space="PSUM"))
        opsp = ctx.enter_context(tc.tile_pool(name="ops", bufs=2, space="PSUM"))

        def emit_rep():
            xf_tiles = {}

            def emit_x_load(sb):
                xf = xfp.tile([P, D], F32, tag="xf")
                nc.sync.dma_start(out=xf, in_=x[sb * P:(sb + 1) * P, :])
                xf_tiles[sb] = xf

            wvr = Wv.rearrange("(c p) e -> p c e", p=P)

            def load_wv_half(h):
                for ci in range(DC):
                    wst = wstgp.tile([P, PG], F32, tag="wst")
                    nc.sync.dma_start(
                        out=wst, in_=wvr[:, ci, h * PG:(h + 1) * PG])
                    nc.scalar.activation(
                        Wv_sb[:, ci, h * PG:(h + 1) * PG], wst,
                        AFT.Identity, scale=1.0)

            def load_w_hslice(W, c, h):
                # e-slice c, d'-half h of Wq/Wk: [P, HDC, P] fp32 -> bf16
                wr = W.rearrange("(c p) e -> p c e", p=P)
                wst = wslp.tile([P, HDC, P], F32, tag="wsl")
                # ACT ring: these have no data waits, so they prefetch in
                # parallel with the sync ring's x stream instead of
                # queueing behind it
                nc.scalar.dma_start(
                    out=wst,
                    in_=wr[:, h * HDC:(h + 1) * HDC, c * P:(c + 1) * P])
                wt = wqkp.tile([P, HDC, P], BF16, tag="wqk")
                nc.scalar.activation(wt, wst, AFT.Identity, scale=1.0)
                return wt

            # single transpose-PSUM generation per rep (W and x phases
            # share it; slice hazards pipeline the drains)
            pst = tpsp.tile([P, DC, P], BF16, tag="pst")

            # ---- Merged phase 0 + stage 1 ----
            # One interleaved pipeline: x transposes and V groups are the
            # PE filler while Wq/Wk slices stream and get transposed
            # (tiny PE), then A, B^T, w. Every engine queue receives its
            # ops in data-arrival order.
            with nc.named_scope("stage1"):
                def emit_transpose(sb):
                    x16 = x16p.tile([P, D], BF16, tag="x16")
                    nc.vector.tensor_copy(x16, xf_tiles.pop(sb))
                    for g in range(2):
                        for c4 in range(4):
                            c = g * 4 + c4
                            nc.tensor.transpose(
                                pst[:, c, :], x16[:, c * P:(c + 1) * P],
                                ident)
                        nc.vector.tensor_copy(
                            xT_sb[:, g * 4:g * 4 + 4, sb * P:(sb + 1) * P],
                            pst[:, g * 4:g * 4 + 4, :])

                def emit_wkt(c, h):
                    wt = load_w_hslice(Wk, c, h)
                    for cd in range(HDC):
                        nc.tensor.transpose(
                            pst[:, cd, :], wt[:, cd, :], ident)
                    nc.vector.tensor_copy(WkT_sb[:, c], pst[:, 0:HDC, :])

                def emit_wqt(c):
                    for h2 in range(2):
                        wt = load_w_hslice(Wq, c, h2)
                        for cd in range(HDC):
                            nc.tensor.transpose(
                                pst[:, h2 * HDC + cd, :], wt[:, cd, :],
                                ident)
                        nc.vector.tensor_copy(
                            WqT_sb[:, c, h2 * HDC:(h2 + 1) * HDC, :],
                            pst[:, h2 * HDC:(h2 + 1) * HDC, :])

                def emit_a(cd, h):
                    ps = workp.tile([P, PG], F32, tag="wps")
                    for ce in range(DC):
                        nc.tensor.matmul(
                            ps, WqT_sb[:, ce, cd, :], WkT_sb[:, ce],
                            start=(ce == 0), stop=(ce == DC - 1))
                    nc.scalar.activation(
                        A_sb[:, cd, h * PG:(h + 1) * PG], ps,
                        AFT.Identity, scale=1.0)

                def emit_t(cd, h):
                    tp = workp.tile([P, PG], F32, tag="wps")
                    for ce in range(DC):
                        nc.tensor.matmul(
                            tp[:, 0:1], WkT_sb[:, ce, cd, :],
                            bq16[:, ce:ce + 1],
                            start=(ce == 0), stop=(ce == DC - 1))
                    nc.vector.tensor_copy(
                        t16[:, h * HDC + cd:h * HDC + cd + 1], tp[:, 0:1])

                def emit_v(sb, h):
                    ps = workp.tile([P, PG], F32, tag="wps")
                    for ci in range(DC):
                        nc.tensor.matmul(
                            ps, xT_sb[:, ci, sb * P:(sb + 1) * P],
                            Wv_sb[:, ci, h * PG:(h + 1) * PG],
                            start=(ci == 0), stop=(ci == DC - 1))
                    nc.vector.tensor_add(
                        V_sb[:, sb, h * PG:(h + 1) * PG], ps,
                        bv_sb[:, h * PG:(h + 1) * PG])

                def emit_bt(cp, g):
                    ps = workp.tile([P, PG], F32, tag="wps")
                    for cd in range(DC):
                        nc.tensor.matmul(
                            ps, A_sb[:, cd, cp * P:(cp + 1) * P],
                            xT_sb[:, cd, g * PG:(g + 1) * PG],
                            start=(cd == 0), stop=(cd == DC - 1))
                    nc.scalar.activation(
                        BT_sb[:, cp, g * PG:(g + 1) * PG], ps,
                        AFT.Identity, scale=1.0)

                def emit_w(sb):
                    wp = workp.tile([P, PG], F32, tag="wps")
                    for ci in range(DC):
                        nc.tensor.matmul(
                            wp[:, 0:1], xT_sb[:, ci, sb * P:(sb + 1) * P],
                            t16[:, ci:ci + 1],
                            start=(ci == 0), stop=(ci == DC - 1))
                    nc.scalar.activation(w_sb[:, sb:sb + 1], wp[:, 0:1],
                                         AFT.Identity, scale=scale)

                # Ring order: x + Wv first (T and V groups keep PE
                # saturated through the whole x stream), then the Wq/Wk
                # slices, whose transposes + A + B^T chains follow as
                # self-paced fillers.
                emit_x_load(0)
                emit_x_load(1)
                emit_transpose(0)
                emit_transpose(1)
                load_wv_half(0)
                emit_x_load(2)
                emit_x_load(3)
                emit_transpose(2)
                emit_transpose(3)
                load_wv_half(1)
                emit_x_load(4)
                emit_x_load(5)
                emit_transpose(4)
                emit_transpose(5)
                emit_v(0, 0)
                emit_v(0, 1)
                vq = [(sb, h) for sb in range(1, SB) for h in range(2)]
                for sb in range(6, SB):
                    emit_x_load(sb)
                    emit_transpose(sb)
                    for _ in range(2):
                        if vq and vq[0][0] < sb:
                            emit_v(*vq.pop(0))
                for c in range(DC):
                    emit_wkt(c, 0)
                    if c % 2 and vq:
                        emit_v(*vq.pop(0))
                for c in range(DC):
                    emit_wqt(c)
                    if c % 2 and vq:
                        emit_v(*vq.pop(0))
                for cd in range(DC):
                    emit_a(cd, 0)
                    if cd % 4 == 3 and vq:
                        emit_v(*vq.pop(0))
                for cd in range(HDC):
                    emit_t(cd, 0)
                for c in range(DC):
                    emit_wkt(c, 1)
                    if c % 2 and vq:
                        emit_v(*vq.pop(0))
                for cd in range(DC):
                    emit_a(cd, 1)
                    if vq:
                        emit_v(*vq.pop(0))
                for cd in range(HDC):
                    emit_t(cd, 1)
                while vq:
                    emit_v(*vq.pop(0))
                # w chains are low-density (rank-1); interleave them
                # behind the dense B^T chains so their latency hides
                for g in range(S // PG):
                    for cp in range(DC):
                        emit_bt(cp, g)
                        if cp % 2:
                            emit_w(g * HDC + cp // 2)

            # ---- Stage 2 ----
            with nc.named_scope("stage2"):
                for qg in range(NQG):
                    PT = ptp.tile([P, SB, PG], BF16, tag="PT")
                    q_lo = qg * PG
                    for kb in range(SB):
                        ps = workp.tile([P, PG], F32, tag="wps")
                        for c in range(DC):
                            nc.tensor.matmul(
                                ps, xT_sb[:, c, kb * P:(kb + 1) * P],
                                BT_sb[:, c, q_lo:q_lo + PG],
                                start=(c == 0), stop=(c == DC - 1))
                        nc.scalar.activation(PT[:, kb, :], ps, AFT.Exp,
                                             bias=w_sb[:, kb:kb + 1],
                                             scale=scale)
                    for qb in range(PG // P):
                        q0 = q_lo + qb * P
                        ps_o = opsp.tile([P, D], F32, tag="ps_o")
                        lt = workp.tile([P, PG], F32, tag="wps")
                        ps_l = lt[:, 0:1]
                        for kb in range(SB):
                            pt_s = PT[:, kb, qb * P:(qb + 1) * P]
                            for h in range(D // PG):
                                nc.tensor.matmul(
                                    ps_o[:, h * PG:(h + 1) * PG], pt_s,
                                    V_sb[:, kb, h * PG:(h + 1) * PG],
                                    start=(kb == 0), stop=(kb == SB - 1))
                            nc.tensor.matmul(ps_l, pt_s, ones,
                                             start=(kb == 0),
                                             stop=(kb == SB - 1))
                        rl = lstp.tile([P, 1], F32, tag="rl")
                        nc.vector.reciprocal(rl, ps_l)
                        for h in range(D // PG):
                            sl = slice(h * PG, (h + 1) * PG)
                            o_t = otp.tile([P, PG], F32, tag="o_t")
                            nc.vector.tensor_scalar_mul(o_t, ps_o[:, sl], rl)
                            nc.scalar.dma_start(out=y[q0:q0 + P, sl], in_=o_t)

        for _rep in range(reps):
            emit_rep()

    nc.compile()
    return nc


_NC_CACHE = {}


def _get_nc():
    if "nc" not in _NC_CACHE:
        _NC_CACHE["nc"] = build_attention_nc(S=S, D=D)
    return _NC_CACHE["nc"]


def run(inputs, trace=False, **run_kwargs):
    """Shard over batch, run on cores 0..7, gather. Returns (y, BassKernelResults)."""
    x = np.ascontiguousarray(np.asarray(inputs["x"], dtype=np.float32))
    shared = {
        k: np.ascontiguousarray(np.asarray(inputs[k], dtype=np.float32))
        for k in ("Wq", "Wk", "Wv", "bq", "bk", "bv")
    }
    in_maps = [dict(shared, x=x[b]) for b in range(B)]
    nc = _get_nc()
    res = run_bass_kernel_spmd(nc, in_maps, core_ids=list(range(N_CORES)),
                               trace=trace, **run_kwargs)
    y = np.stack([res.results[b]["y"] for b in range(B)], axis=0)
    return y, res


def kernel(**inputs):
    y, _ = run(inputs, trace=False)
    return y
